# revision 7
# baseline (speedup 1.0000x reference)
"""Trainium2 Bass kernel for nn_MemoryGraphBackprop (GNN message passing).

Strategy (v2, transpose-free)
-----------------------------
T=64 sequential steps over state [BS=2, N=1024, D=64].  All operands live
in SBUF on ONE NeuronCore (a per-step 8-core shard was investigated: the
required per-step cross-core exchange is impossible here — SWDGE remote-DMA
desc-gen ucode faults in this environment and collective_compute costs
~8.9us per call x 64 sequential steps).

Math per step t (u := prim*h):
    r  = A @ pm  (+ cc_t into nodes < C, folded as +w2*cc)
    u' = dt*u + w2*r,   w2 = (1-dt)*prim,  dt = decay*(1-eot[b,t])
    pm' = tanh(u')

Layout: everything in "layout-1" [n%128 partitions, b*64+d free].  The
matmul computes r^T directly in that layout:
    out[p=n, f=bd] = sum_m lhsT[m, n] * rhs[m, bd],
lhsT = A^T block (static! prefetchable stationary), rhs = pm chunk m.
64 matmuls of FD=128 per step (8 n-blocks x 8 m-chunks), accumulated in 8
PSUM tiles — and NO transposes at all: the chain + tanh emit pm' already
in matmul-operand layout.  n-block-OUTER order makes block 0 finish its
accumulation 1/8 into the step, so its chain/tanh (DVE+ACT) overlap the
remaining matmuls and the next step's m=0 matmuls never wait.

dt is baked per step: eot is known at build time, so the four (eot_b0,
eot_b1) variants of w2 and dt are HOST-precomputed tensors and each step
just picks its variant — handles non-uniform decay for free.  State u is
kept in fp32 (su = dt*u runs on gpsimd off the critical path); only pm and
matmul operands are bf16.

Output: pm'[nodes<64] = psum block 0 partitions 0-63 — emitted per step by
a second fp32 tanh, again with no transpose.
"""

import sys

if "/opt/trn_rl_repo" not in sys.path:
    sys.path.insert(0, "/opt/trn_rl_repo")

import numpy as np

import concourse.bass as bass
import concourse.mybir as mybir
import concourse.tile as tile
from concourse import bass_utils

BS, T, C, D = 2, 64, 64, 64
N = 1024
NB = N // 128   # 8 node blocks (also 8 contraction chunks)
P = 128         # b*64+d free width

F32 = mybir.dt.float32
BF16 = mybir.dt.bfloat16

# ---------------------------------------------------------------------------
# Workaround: this container's walrus accepts only ONE sync-wait per
# instruction.  (1) Tile's tail drain attaches one wait per live semaphore —
# split across multiple drains.  (2) Any multi-wait instruction gets its
# extra waits hoisted onto InstEventSemaphore carriers just before it.
# ---------------------------------------------------------------------------
from concourse.vector_clock import ScopedClock  # noqa: E402


def _patched_drain_and_barrier(self, tick_clock, wait_clock):
    drain_inst = self.nc.sync.drain()
    wait_clock.add_sem_waits(
        drain_inst.ins, ScopedClock({None: tick_clock.global_clock})
    )
    si = drain_inst.ins.sync_info
    if si is not None and si.on_wait is not None and len(si.on_wait) > 1:
        waits = list(si.on_wait)
        drain_inst.ins.sync_info = mybir.SyncInfo(
            on_wait=[waits[0]], on_update=si.on_update
        )
        for w in waits[1:]:
            d2 = self.nc.sync.drain()
            d2.ins.sync_info = mybir.SyncInfo(on_wait=[w], on_update=[])

    self.nc.all_engine_barrier()
    assert self.sems is not None
    popped = self.nc._tile_sem_poison_stack.pop()
    assert popped is self._sem_poison
    self.nc.clear_and_free_semaphores(list(self.sems.allocated().values()))
    self.nc.all_engine_barrier()


tile.TileContext._drain_and_barrier = _patched_drain_and_barrier


def _split_multi_waits(nc):
    n_carriers = 0
    for bb in nc.m.functions[0].blocks:
        insts = list(bb.instructions)
        out = []
        changed = False
        for inst in insts:
            si = inst.sync_info
            if si is not None and si.on_wait is not None and len(si.on_wait) > 1:
                waits = list(si.on_wait)
                for w in waits[:-1]:
                    n_carriers += 1
                    carrier = mybir.InstEventSemaphore(
                        name=f"waitsplit-{n_carriers}", ins=[], outs=[]
                    )
                    carrier.engine = inst.engine
                    carrier.sync_info = mybir.SyncInfo(on_wait=[w], on_update=[])
                    out.append(carrier)
                inst.sync_info = mybir.SyncInfo(
                    on_wait=[waits[-1]], on_update=si.on_update
                )
                changed = True
            out.append(inst)
        if changed:
            bb.instructions = out
    return n_carriers


# ---------------------------------------------------------------------------
# Host-side input massaging.
# ---------------------------------------------------------------------------
def _prep_host(inputs):
    import ml_dtypes

    bf16 = ml_dtypes.bfloat16

    cc = np.asarray(inputs["cc_signals"], dtype=np.float32)       # [B,T,C,D]
    eot = np.asarray(inputs["eot_mask"]).astype(bool)             # [B,T]
    idx = np.asarray(inputs["conn_indices"]).astype(np.int64)     # [N,K]
    cmask = np.asarray(inputs["conn_mask"]).astype(np.float32)    # [N,K]
    prim = np.asarray(inputs["primitives"], dtype=np.float32)     # [N,D]
    w = np.asarray(inputs["conn_weights"], dtype=np.float32)      # [N,K]
    dlog = np.asarray(inputs["decay_logit"], dtype=np.float32)    # [N]
    h0 = np.asarray(inputs["h0"], dtype=np.float32)               # [B,N,D]
    pm0 = np.asarray(inputs["prev_msg0"], dtype=np.float32)       # [B,N,D]

    # dense adjacency; lhsT block (nb, m) = At[m*128:(m+1)*128, nb*128:+128]
    # stored in DMA==consumption order: block index j = nb*8 + m.
    A = np.zeros((N, N), dtype=np.float32)
    np.add.at(A, (np.arange(N)[:, None], idx), w * cmask)
    At = A.T                                                     # [m, n]
    at_host = np.ascontiguousarray(
        At.reshape(NB, 128, NB, 128).transpose(1, 2, 0, 3).reshape(128, NB * NB * 128)
    )  # [k, (nb, m, p)]

    decay = (1.0 / (1.0 + np.exp(-dlog.astype(np.float64)))).astype(np.float32)

    # layout-1 helpers: X[n, b*64+d] from [B, N, D]
    def l1(x_bnd):
        return np.ascontiguousarray(
            x_bnd.transpose(1, 0, 2).reshape(N, BS * D)
        )

    prim_l1 = np.ascontiguousarray(
        np.broadcast_to(prim[None], (BS, N, D)).transpose(1, 0, 2).reshape(N, P)
    )  # [n, bd] (d part repeats per b)

    # four eot variants: v = eot_b0*2 + eot_b1; dt_v[n, bd] = decay[n]*(1-v_b)
    w2_v = np.empty((4, N, P), dtype=np.float32)
    dt_v = np.empty((4, N, P), dtype=np.float32)
    for v in range(4):
        e = np.array([(v >> 1) & 1, v & 1], dtype=np.float32)    # [b0, b1]
        live = (1.0 - e)[:, None] * np.ones((BS, D), dtype=np.float32)
        live_bd = live.reshape(P)                                # [bd]
        dt = decay[:, None] * live_bd[None, :]                   # [n, bd]
        dt_v[v] = dt
        w2_v[v] = (1.0 - dt) * prim_l1

    vt = (eot[0].astype(np.int64) << 1) | eot[1].astype(np.int64)  # [T]

    # cc: L2-normalize, then cw_t = w2_{v(t)}[:C] * ccn_t  [C, bd] per step
    nrm = np.maximum(np.linalg.norm(cc, axis=-1, keepdims=True), 1e-8)
    ccn = cc / nrm                                               # [B,T,C,D]
    ccn_l1 = ccn.transpose(2, 1, 0, 3).reshape(C, T, P)          # [c, t, bd]
    cw = np.empty((C, T, P), dtype=np.float32)
    for t in range(T):
        cw[:, t, :] = w2_v[vt[t]][:C] * ccn_l1[:, t, :]
    cw_host = np.ascontiguousarray(cw.reshape(C, T * P))

    u0 = prim_l1 * l1(h0)                                        # [n, bd] f32
    su0 = dt_v[vt[0]] * u0                                       # [n, bd] f32
    pm0_l1 = l1(pm0)                                             # [n, bd]

    def blk128(x, dtype):  # [N, P] -> [128, NB*P] (block-major free)
        return np.ascontiguousarray(
            x.reshape(NB, 128, P).transpose(1, 0, 2).reshape(128, NB * P)
        ).astype(dtype)

    host = {
        "at": at_host.astype(bf16),
        "cw": cw_host.astype(bf16),
        "su0": blk128(su0, np.float32),
        "pm0": blk128(pm0_l1, bf16),
    }
    for v in range(4):
        host[f"w2v{v}"] = blk128(w2_v[v], bf16)
        host[f"dtv{v}"] = blk128(dt_v[v], bf16)
    return host, vt


# ---------------------------------------------------------------------------
# Device kernel (module depends on the per-step eot variant sequence vt).
# ---------------------------------------------------------------------------
def _build_bass(vt):
    nc = bass.Bass("TRN2", target_bir_lowering=False, debug=False)

    at_d = nc.dram_tensor("at", [128, NB * NB * 128], BF16, kind="ExternalInput")
    cw_d = nc.dram_tensor("cw", [C, T * P], BF16, kind="ExternalInput")
    su0_d = nc.dram_tensor("su0", [128, NB * P], F32, kind="ExternalInput")
    pm0_d = nc.dram_tensor("pm0", [128, NB * P], BF16, kind="ExternalInput")
    w2_d = [nc.dram_tensor(f"w2v{v}", [128, NB * P], BF16, kind="ExternalInput")
            for v in range(4)]
    dt_d = [nc.dram_tensor(f"dtv{v}", [128, NB * P], BF16, kind="ExternalInput")
            for v in range(4)]
    out_d = nc.dram_tensor("out", [T, C, P], F32, kind="ExternalOutput")

    Tanh = mybir.ActivationFunctionType.Tanh
    vused = sorted(set(int(v) for v in vt))

    with tile.TileContext(nc) as tc:
        with (
            tc.tile_pool(name="consts", bufs=1) as consts,
            tc.tile_pool(name="state", bufs=3) as state,
            tc.tile_pool(name="tmp", bufs=4) as tmp,
            tc.tile_pool(name="psr", bufs=2, space="PSUM") as psr,
        ):
            # HAM warm-up: dummy matmuls keep the PE activity monitor at
            # full clock while the input DMAs land.
            from concourse.masks import make_identity
            id128_sb = consts.tile([128, 128], BF16)
            make_identity(nc, id128_sb[:])
            warm_ps = psr.tile([128, 128], F32, tag="warm", name="warm_ps")
            for i in range(48):
                nc.tensor.matmul(
                    warm_ps[:], id128_sb[:], id128_sb[:],
                    start=(i == 0), stop=(i == 47), skip_group_check=True,
                )

            # --- small state first so step-0 deps clear early ---
            pm = [state.tile([128, P], BF16, tag=f"pm{m}", name=f"pm{m}")
                  for m in range(NB)]
            su = [state.tile([128, P], F32, tag=f"su{m}", name=f"su{m}")
                  for m in range(NB)]
            for m in range(NB):
                nc.sync.dma_start(out=pm[m][:], in_=pm0_d.ap()[:, m * P:(m + 1) * P])
                nc.sync.dma_start(out=su[m][:], in_=su0_d.ap()[:, m * P:(m + 1) * P])

            # big static slabs; at blocks stream in consumption order
            at_sb = consts.tile([128, NB * NB * 128], BF16)
            for j in range(NB * NB):
                sl = slice(j * 128, (j + 1) * 128)
                nc.sync.dma_start(out=at_sb[:, sl], in_=at_d.ap()[:, sl])
            w2_sb = {}
            dt_sb = {}
            for v in vused:
                w2_sb[v] = consts.tile([128, NB * P], BF16, name=f"w2sb{v}")
                nc.sync.dma_start(out=w2_sb[v][:], in_=w2_d[v].ap()[:])
            for v in vused:
                dt_sb[v] = consts.tile([128, NB * P], BF16, name=f"dtsb{v}")
                nc.sync.dma_start(out=dt_sb[v][:], in_=dt_d[v].ap()[:])
            cw_sb = consts.tile([C, T * P], BF16)
            for q in range(4):
                sl = slice(q * (T * P) // 4, (q + 1) * (T * P) // 4)
                nc.sync.dma_start(out=cw_sb[:, sl], in_=cw_d.ap()[:, sl])

            for t in range(T):
                v = int(vt[t])
                vn = int(vt[t + 1]) if t + 1 < T else v
                # 4 n-blocks share one PSUM bank tile (4 independent
                # accumulation groups over disjoint 128-col slices)
                psh = [psr.tile([128, 4 * P], F32, tag=f"psh{h}", name=f"psh{h}")
                       for h in range(2)]
                ps = [psh[nb // 4][:, (nb % 4) * P:(nb % 4 + 1) * P]
                      for nb in range(NB)]
                pmn = [state.tile([128, P], BF16, tag=f"pm{nb}", name=f"pmn{nb}")
                       for nb in range(NB)]
                sun = [state.tile([128, P], F32, tag=f"su{nb}", name=f"sun{nb}")
                       for nb in range(NB)]

                for nb in range(NB):
                    for m in range(NB):
                        nc.tensor.matmul(
                            ps[nb],
                            at_sb[:, (nb * NB + m) * 128:(nb * NB + m + 1) * 128],
                            pm[m][:],
                            start=(m == 0), stop=(m == NB - 1),
                            skip_group_check=True,
                        )
                    # chain for block nb, right after its matmuls: overlaps
                    # the remaining blocks' matmuls.
                    x = tmp.tile([128, P], F32, tag=f"x{nb}", name=f"x{nb}")
                    w2s = w2_sb[v][:, nb * P:(nb + 1) * P]
                    nc.vector.tensor_mul(x[:], ps[nb], w2s)
                    if nb == 0:
                        nc.vector.tensor_add(
                            x[0:C], x[0:C], cw_sb[:, t * P:(t + 1) * P]
                        )
                    up = tmp.tile([128, P], F32, tag=f"up{nb}", name=f"up{nb}")
                    nc.vector.tensor_add(up[:], x[:], su[nb][:])
                    if t < T - 1:
                        nc.scalar.activation(pmn[nb][:], up[:], Tanh)
                        # su for next step on gpsimd: off the critical path
                        nc.gpsimd.tensor_mul(
                            sun[nb][:], up[:], dt_sb[vn][:, nb * P:(nb + 1) * P]
                        )
                    if nb == 0:
                        out_sb = tmp.tile([C, P], F32, tag="osb", name="osb")
                        nc.scalar.activation(out_sb[:], up[0:C], Tanh)
                        nc.sync.dma_start(out=out_d.ap()[t], in_=out_sb[:])

                pm, su = pmn, sun

    _split_multi_waits(nc)
    return nc


RUN_KWARGS: dict = {}
_BUILT: dict = {}


def _get_built(vt):
    key = tuple(int(x) for x in vt)
    if key not in _BUILT:
        _BUILT[key] = _build_bass(vt)
    return _BUILT[key]


def kernel(**inputs) -> np.ndarray:
    host, vt = _prep_host(inputs)
    nc = _get_built(vt)
    res = bass_utils.run_bass_kernel_spmd(nc, [host], core_ids=[0], **RUN_KWARGS)
    kernel.last_result = res
    out_dev = res.results[0]["out"]                               # [T, C, 128]
    out = out_dev.reshape(T, C, BS, D).transpose(2, 0, 1, 3)      # [B,T,C,D]
    return np.ascontiguousarray(out)


if __name__ == "__main__":
    print("standalone smoke: building bass module...")
    _get_built(np.zeros(T, dtype=np.int64))
    print("built ok")
